# revision 1
# baseline (speedup 1.0000x reference)
"""RNN-T decoder (embedding + 2-layer LSTM + joint network) on 8 Trainium2 cores.

Strategy:
  - LSTM (B=4, U=64, D=1024) is inherently sequential with a tiny batch; it runs
    redundantly on all 8 cores near its PE floor. Input projections are batched
    over all steps; the per-step recurrent matmul streams W_hh through the PE
    with h^T as the (tiny) stationary operand.
  - Joint network (dominant FLOPs) is sharded over T: core c computes
    out[:, c*16:(c+1)*16, :, :].
  - All weights are pre-transposed on the host so DMA loads are contiguous.

kernel(**inputs) takes the full unsharded inputs (as in reference.setup_inputs)
and returns the full (B, T, U, ODIM) float32 output.
"""
import sys
import numpy as np

sys.path.insert(0, "/opt/trn_rl_repo")

import concourse.bass as bass
import concourse.bacc as bacc
import concourse.mybir as mybir
import concourse.tile as tile
from concourse.bass_utils import run_bass_kernel_spmd
from concourse.masks import make_identity
from contextlib import ExitStack

F32 = mybir.dt.float32
F32R = mybir.dt.float32r
BF16 = mybir.dt.bfloat16
I32 = mybir.dt.int32
AF = mybir.ActivationFunctionType
OP = mybir.AluOpType

B, T, U = 4, 128, 64
NCORES = 8
TC = T // NCORES          # 16 T-columns per core
E, D, G = 512, 1024, 4096  # embed, dunits, 4*dunits
J, O = 512, 2048           # joint dim, odim
UB = U * B                 # 256, u-major token index
BT = B * TC                # 64 encoder rows per core
NB = G // 512              # 8 gate blocks of 512
KD = D // 128              # 8 contraction chunks of hidden dim
# gate block order for streaming: g first, then f, i, o (c-chain starts early,
# o last since its only consumer is the final h multiply)
NBORDER = [4, 5, 2, 3, 0, 1, 6, 7]

_CACHE = {}


class _SkipJoint(Exception):
    pass


def _mm_r(nc, out, lhsT, rhs, **kw):
    """matmul with both operands viewed as float32r (full-rate fp32 storage)."""
    nc.tensor.matmul(out, lhsT=lhsT.bitcast(F32R), rhs=rhs.bitcast(F32R), **kw)


def _emit_xproj(nc, pools, rhs_of_ec, w_of_ec, nchunks,
                bih, bhh, gxT, accum, with_bias):
    """gxT[gm][:, :] (+)= (W @ x^T) block + bias, for 32 g-tiles of 128.

    rhs_of_ec: ec -> [128, 256] rhs tile (x^T chunk, K on partitions)
    w_of_ec:   ec -> [128, G] tile of W.T rows for that chunk (lhsT source)
    bih/bhh: DRAM bias handles (summed on device), used when with_bias
    accum: False -> overwrite gxT, True -> add into gxT
    """
    pbig, bpool = pools["pbig"], pools["bias"]
    ones_r = pools["ones"]
    for gm in range(32):
        gs = slice(128 * (gm % 4), 128 * (gm % 4) + 128)
        if with_bias and gm % 4 == 0:
            cb = slice((gm // 4) * 512, (gm // 4) * 512 + 512)
            ba = bpool.tile([1, 512], F32R, tag="ba", name="ba")
            bb = bpool.tile([1, 512], F32, tag="bb", name="bb")
            nc.sync.dma_start(ba[:1, :], bih[None, cb].bitcast(F32R))
            nc.sync.dma_start(bb[:1, :], bhh[None, cb])
            nc.vector.tensor_tensor(ba[:1, :], in0=ba[:1, :],
                                    in1=bb[:1, :], op=OP.add)
        ps = pbig.tile([128, 256], F32, tag="pbig", name="pbig")
        for ec in range(nchunks):
            _mm_r(nc, ps[:],
                  lhsT=w_of_ec(ec)[:, 128 * gm: 128 * (gm + 1)],
                  rhs=rhs_of_ec(ec),
                  start=(ec == 0), stop=(ec == nchunks - 1 and not with_bias))
        if with_bias:
            _mm_r(nc, ps[:], lhsT=ba[:1, gs], rhs=ones_r[:1, :256],
                  start=False, stop=True)
        if accum:
            nc.vector.tensor_tensor(gxT[gm][:], in0=gxT[gm][:], in1=ps[:],
                                    op=OP.add)
        else:
            nc.vector.tensor_copy(gxT[gm][:], ps[:])


def _emit_lstm_layer(nc, pools, ident, whh, gxT, hT_all, c_sb, gates, hbuf,
                     steps):
    """One LSTM layer, `steps` sequential steps.

    whh: 8 SBUF tiles [128, G] = W_hh.T chunks (rhs stream)
    gxT: 32 SBUF tiles [128, 256] = batched (W_ih x + bias)^T, cols ub = 4u+b
    hT_all: 8 SBUF tiles [128, 4*(U+1)]; col block u = h[u-1].T chunk
            (block 0 = zeros); this layer writes block u+1.
    """
    pgates, ptr = pools["pgates"], pools["ptr"]
    nborder = pools.get("nborder") or NBORDER
    ident4 = ident[:4, :4]
    for u in range(steps):
        stat = [hT_all[k][:, 4 * u: 4 * u + 4] for k in range(KD)]
        us = slice(4 * u, 4 * u + 4)
        for nb in nborder:
            nbs = slice(nb * 512, (nb + 1) * 512)
            ps = pgates.tile([4, 512], F32, tag="pg", name="pg")
            # inject the batched x-projection first (ps[:, 128c:] = gxT[...].T):
            # it depends only on gxT, so the PE can start it before this
            # step's h^T copies land
            identb = pools["identb"]
            if pools.get("inject_first", True):
                for c in range(4):
                    nc.tensor.matmul(ps[:, 128 * c:128 * (c + 1)],
                                     lhsT=gxT[4 * nb + c][:, us], rhs=identb[:],
                                     start=(c == 0), stop=False)
                for k in range(KD):
                    _mm_r(nc, ps[:], lhsT=stat[k], rhs=whh[k][:, nbs],
                          start=False, stop=(k == KD - 1))
            else:
                for k in range(KD):
                    _mm_r(nc, ps[:], lhsT=stat[k], rhs=whh[k][:, nbs],
                          start=(k == 0), stop=False)
                for c in range(4):
                    nc.tensor.matmul(ps[:, 128 * c:128 * (c + 1)],
                                     lhsT=gxT[4 * nb + c][:, us], rhs=identb[:],
                                     start=False, stop=(c == 3))
            if pools.get("tanh_only") and nb not in (4, 5):
                # sigmoid(x) = 0.5*tanh(x/2) + 0.5 — keeps ACT on one table set
                nc.scalar.activation(gates[:, nbs], ps[:], AF.Tanh, scale=0.5)
                nc.vector.tensor_scalar(gates[:, nbs], gates[:, nbs], 0.5, 0.5,
                                        OP.mult, OP.add)
            else:
                fn = pools.get("act_fn") or (AF.Tanh if nb in (4, 5) else AF.Sigmoid)
                nc.scalar.activation(gates[:, nbs], ps[:], fn)
        # c' = sig(f)*c + sig(i)*tanh(g);  h = sig(o)*tanh(c')
        # transposes of each 512-half issue as soon as that half of h is ready
        for hh in range(2):
            s = slice(hh * 512, (hh + 1) * 512)
            gi = gates[:, 0 * D:][:, s]
            gf = gates[:, 1 * D:][:, s]
            gg = gates[:, 2 * D:][:, s]
            go = gates[:, 3 * D:][:, s]
            ch = c_sb[:, s]
            nc.vector.tensor_tensor(gf, in0=gf, in1=ch, op=OP.mult)      # f*c
            nc.vector.tensor_tensor(gi, in0=gi, in1=gg, op=OP.mult)      # i*g
            nc.vector.tensor_tensor(ch, in0=gi, in1=gf, op=OP.add)       # c'
            nc.scalar.activation(gg, ch, pools.get("act_fn") or AF.Tanh)
            nc.vector.tensor_tensor(hbuf[:, s], in0=go, in1=gg, op=OP.mult)
            for k in range(4 * hh, 4 * hh + 4):
                tp = ptr.tile([128, 4], F32, tag="tr", name="tr")
                nc.tensor.transpose(tp[:], in_=hbuf[:, 128 * k: 128 * (k + 1)],
                                    identity=ident4)
                eng = pools.get("ht_engine", "vector")
                getattr(nc, eng).tensor_copy(
                    hT_all[k][:, 4 * (u + 1): 4 * (u + 1) + 4], tp[:])


def build_nc(steps=U, layers=2, joint=True, act_fn=None, tanh_only=False,
             pg_bufs=3, ptr_bufs=2, ht_engine='vector', inject_first=True,
             nborder=None):
    nc = bacc.Bacc("TRN2", target_bir_lowering=False, debug=False)

    hs = nc.dram_tensor("hs", [BT, E], F32, kind="ExternalInput")
    ys_idx = nc.dram_tensor("ys_idx", [UB], I32, kind="ExternalInput")
    embed = nc.dram_tensor("embed", [O, E], F32, kind="ExternalInput")
    wih0T = nc.dram_tensor("wih0T", [E, G], F32R, kind="ExternalInput")
    whh0T = nc.dram_tensor("whh0T", [D, G], F32R, kind="ExternalInput")
    wih1T = nc.dram_tensor("wih1T", [D, G], F32R, kind="ExternalInput")
    whh1T = nc.dram_tensor("whh1T", [D, G], F32R, kind="ExternalInput")
    bih0 = nc.dram_tensor("bih0", [G], F32, kind="ExternalInput")
    bhh0 = nc.dram_tensor("bhh0", [G], F32, kind="ExternalInput")
    bih1 = nc.dram_tensor("bih1", [G], F32, kind="ExternalInput")
    bhh1 = nc.dram_tensor("bhh1", [G], F32, kind="ExternalInput")
    wencT = nc.dram_tensor("wencT", [E, J], F32R, kind="ExternalInput")
    wdecT = nc.dram_tensor("wdecT", [D, J], F32R, kind="ExternalInput")
    woutT = nc.dram_tensor("woutT", [J, O], F32R, kind="ExternalInput")
    benc = nc.dram_tensor("benc", [J], F32R, kind="ExternalInput")
    bout_bc = nc.dram_tensor("bout_bc", [128, O], F32, kind="ExternalInput")
    ones_d = nc.dram_tensor("ones_d", [256], F32R, kind="ExternalInput")
    ident_f = nc.dram_tensor("ident_f", [128, 128], F32, kind="ExternalInput")
    ident_b = nc.dram_tensor("ident_b", [128, 128], BF16, kind="ExternalInput")
    out = nc.dram_tensor("out", [BT * U, O], F32, kind="ExternalOutput")

    with tile.TileContext(nc) as tc, ExitStack() as es:
        cpool = es.enter_context(tc.tile_pool(name="const", bufs=1))
        ppool = es.enter_context(tc.tile_pool(name="persist", bufs=1))

        ident = cpool.tile([128, 128], F32, tag="ident", name="ident")
        nc.sync.dma_start(ident[:], ident_f[:])
        ones_r = cpool.tile([1, 256], F32R, tag="ones", name="ones")
        nc.sync.dma_start(ones_r[:1, :], ones_d[None, :])
        identb = cpool.tile([128, 128], BF16, tag="identb", name="identb")
        nc.sync.dma_start(identb[:], ident_b[:])

        gxT = [ppool.tile([128, 256], BF16, tag=f"gxT{g}", name=f"gxT{g}")
               for g in range(32)]
        hT0 = [ppool.tile([128, 4 * (U + 1)], F32R, tag=f"hT0_{k}", name=f"hT0_{k}")
               for k in range(KD)]
        hT1 = [ppool.tile([128, 4 * (U + 1)], F32R, tag=f"hT1_{k}", name=f"hT1_{k}")
               for k in range(KD)]
        gates = ppool.tile([4, G], F32, tag="gates", name="gates")
        c_sb = ppool.tile([4, D], F32, tag="c", name="c")
        hbuf = ppool.tile([4, D], F32, tag="h", name="h")
        for k in range(KD):
            nc.gpsimd.memset(hT0[k][:].bitcast(F32), 0.0)
            nc.gpsimd.memset(hT1[k][:].bitcast(F32), 0.0)
        nc.gpsimd.memset(c_sb[:], 0.0)

        pools = {"ones": ones_r, "act_fn": act_fn, "identb": identb,
                 "tanh_only": tanh_only, "ht_engine": ht_engine,
                 "inject_first": inject_first, "nborder": nborder}

        wenc = [ppool.tile([128, J], F32R, tag=f"wenc{ec}", name=f"wenc{ec}")
                for ec in range(4)]
        benc_sb = ppool.tile([1, J], F32R, tag="benc", name="benc")
        hs_sb = ppool.tile([BT, E], F32, tag="hs_sb", name="hs_sb")
        hsT = [ppool.tile([128, BT], F32R, tag=f"hsT{ec}", name=f"hsT{ec}")
               for ec in range(4)]
        encp = [ppool.tile([128, BT], F32, tag=f"encp{jt}", name=f"encp{jt}")
                for jt in range(4)]

        # ---- Phase 1+2: embedding gather -> eys^T, layer-0 x-projection ----
        with tc.tile_pool(name="ph2", bufs=1) as p2, \
             tc.tile_pool(name="bias2", bufs=2) as bpool2, \
             tc.tile_pool(name="pbig", bufs=4, space="PSUM") as pbig, \
             tc.tile_pool(name="pT", bufs=2, space="PSUM") as pT:
            pools["pbig"] = pbig
            pools["bias"] = bpool2
            # issue the tiny idx DMAs before the 8MB W_ih0 load: they gate
            # the (SWDGE) embedding gather and the first PE transposes
            eysT = [p2.tile([128, 256], F32R, tag=f"eysT{ec}", name=f"eysT{ec}") for ec in range(4)]
            idxs = []
            for t in range(2):
                idx = p2.tile([128, 1], I32, tag=f"idx{t}", name=f"idx{t}")
                nc.sync.dma_start(idx[:, :1], ys_idx[128 * t:128 * (t + 1), None])
                idxs.append(idx)
            wih0 = [p2.tile([128, G], F32R, tag=f"wih0_{ec}", name=f"wih0_{ec}") for ec in range(4)]
            for ec in range(4):
                nc.sync.dma_start(wih0[ec][:], wih0T[128 * ec:128 * (ec + 1), :])
            for t in range(2):
                idx = idxs[t]
                ey = p2.tile([128, E], F32, tag=f"ey{t}", name=f"ey{t}")
                nc.gpsimd.indirect_dma_start(
                    out=ey[:], out_offset=None, in_=embed[:],
                    in_offset=bass.IndirectOffsetOnAxis(ap=idx[:, :1], axis=0))
                for ec in range(4):
                    tp = pT.tile([128, 128], F32, tag="pT", name="pT")
                    nc.tensor.transpose(tp[:], in_=ey[:, 128 * ec:128 * (ec + 1)],
                                        identity=ident[:])
                    nc.vector.tensor_copy(eysT[ec][:, 128 * t:128 * (t + 1)], tp[:])
            _emit_xproj(nc, pools, lambda ec: eysT[ec][:], lambda ec: wih0[ec],
                        4, bih0, bhh0, gxT, accum=False, with_bias=True)

            # encoder-side joint work: hs^T and enc_p^T (PE has slack here)
            for ec in range(4):
                nc.sync.dma_start(wenc[ec][:], wencT[128 * ec:128 * (ec + 1), :])
            nc.sync.dma_start(hs_sb[:], hs[:])
            nc.sync.dma_start(benc_sb[:1, :], benc[None, :])
            for ec in range(4):
                tp = pT.tile([128, 128], F32, tag="pT", name="pT")
                nc.tensor.transpose(tp[:, :BT],
                                    in_=hs_sb[:, 128 * ec:128 * (ec + 1)],
                                    identity=ident[:BT, :BT])
                nc.vector.tensor_copy(hsT[ec][:], tp[:, :BT])
            for jt in range(4):
                tpp = pT.tile([128, 128], F32, tag="pT", name="pT")
                pse = tpp[:, :BT]
                for ec in range(4):
                    _mm_r(nc, pse, lhsT=wenc[ec][:, 128 * jt:128 * (jt + 1)],
                          rhs=hsT[ec][:], start=(ec == 0), stop=False)
                _mm_r(nc, pse, lhsT=benc_sb[:1, 128 * jt:128 * (jt + 1)],
                      rhs=ones_r[:1, :BT], start=False, stop=True)
                nc.vector.tensor_copy(encp[jt][:], pse)

        # ---- Phase 3: layer-0 recurrence ----
        with tc.tile_pool(name="whhA", bufs=1) as whhp, \
             tc.tile_pool(name="pgatesA", bufs=pg_bufs, space="PSUM") as pgates, \
             tc.tile_pool(name="ptrA", bufs=ptr_bufs, space="PSUM") as ptr:
            pools["pgates"], pools["ptr"] = pgates, ptr
            whh = [whhp.tile([128, G], F32R, tag=f"whh{k}", name=f"whh{k}") for k in range(KD)]
            for k in range(KD):
                nc.sync.dma_start(whh[k][:], whh0T[128 * k:128 * (k + 1), :])
            _emit_lstm_layer(nc, pools, ident[:], whh, gxT, hT0, c_sb,
                             gates, hbuf, steps)

        # ---- Phase 4: layer-1 x-projection (streams W_ih1, accumulates) ----
        with tc.tile_pool(name="ph4", bufs=1) as p4, \
             tc.tile_pool(name="bias4", bufs=2) as bpool4, \
             tc.tile_pool(name="pbig2", bufs=4, space="PSUM") as pbig2:
            pools["pbig"] = pbig2
            pools["bias"] = bpool4
            wbuf = [p4.tile([128, G], F32R, tag=f"wih1_{i}", name=f"wih1_{i}") for i in range(4)]
            for p in range(4):
                for i in range(2):
                    k = 2 * p + i
                    nc.sync.dma_start(wbuf[(2 * p + i) % 4][:],
                                      wih1T[128 * k:128 * (k + 1), :])
                _emit_xproj(nc, pools,
                        lambda i, p=p: hT0[2 * p + i][:, 4:4 + 4 * U],
                        lambda i, p=p: wbuf[(2 * p + i) % 4], 2, bih1, bhh1, gxT,
                        accum=(p > 0), with_bias=(p == 3))

        # ---- Phase 5: layer-1 recurrence ----
        nc.gpsimd.memset(c_sb[:], 0.0)
        if layers < 2:
            for k in range(KD):
                nc.vector.tensor_copy(hT1[k][:, 4:4 + 4 * steps],
                                      hT0[k][:, 4:4 + 4 * steps])
        with tc.tile_pool(name="whhB", bufs=1) as whhp2, \
             tc.tile_pool(name="pgatesB", bufs=pg_bufs, space="PSUM") as pgates2, \
             tc.tile_pool(name="ptrB", bufs=ptr_bufs, space="PSUM") as ptr2:
            pools["pgates"], pools["ptr"] = pgates2, ptr2
            if layers >= 2:
                whh2 = [whhp2.tile([128, G], F32R, tag=f"whh2_{k}", name=f"whh2_{k}") for k in range(KD)]
                for k in range(KD):
                    nc.sync.dma_start(whh2[k][:], whh1T[128 * k:128 * (k + 1), :])
                _emit_lstm_layer(nc, pools, ident[:], whh2, gxT, hT1, c_sb,
                                 gates, hbuf, steps)

        # ---- Phase 6: joint network on this core's T-slice ----
        if not joint:
            # still must write the output: cheap memset-like DMA from gates
            zsrc = ppool.tile([128, 512], F32, tag="zsrc", name="zsrc")
            nc.gpsimd.memset(zsrc[:], 0.0)
            for m0 in range(BT * U // 128):
                for ob in range(4):
                    nc.sync.dma_start(out[128 * m0:128 * (m0 + 1),
                                          ob * 512:(ob + 1) * 512], zsrc[:])
        import contextlib
        with contextlib.suppress(_SkipJoint), \
             tc.tile_pool(name="joint", bufs=1) as jp, \
             tc.tile_pool(name="zt", bufs=4) as ztp, \
             tc.tile_pool(name="osb", bufs=4) as osbp, \
             tc.tile_pool(name="pj", bufs=4, space="PSUM") as pj, \
             tc.tile_pool(name="pT2", bufs=2, space="PSUM") as pT2:
            if not joint:
                raise _SkipJoint
            wdec = [jp.tile([128, J], F32R, tag=f"wdec{k}", name=f"wdec{k}") for k in range(KD)]
            wout = [jp.tile([128, O], F32R, tag=f"wout{jt}", name=f"wout{jt}") for jt in range(4)]
            bout_sb = jp.tile([128, O], F32, tag="bout", name="bout")
            decp = [jp.tile([128, 256], F32, tag=f"decp{jt}", name=f"decp{jt}") for jt in range(4)]
            for k in range(KD):
                nc.sync.dma_start(wdec[k][:], wdecT[128 * k:128 * (k + 1), :])
            nc.sync.dma_start(bout_sb[:], bout_bc[:])
            for jt in range(4):
                nc.sync.dma_start(wout[jt][:], woutT[128 * jt:128 * (jt + 1), :])

            # dec_p^T[jt], columns reordered (b, u)
            for jt in range(4):
                ps = pj.tile([128, 256], F32, tag="pj", name="pj")
                for k in range(KD):
                    rhs = hT1[k][:, 4:4 + 4 * U].rearrange("p (u b) -> p b u",
                                                           u=U, b=B)
                    _mm_r(nc, ps[:], lhsT=wdec[k][:, 128 * jt:128 * (jt + 1)],
                          rhs=rhs, start=(k == 0), stop=(k == KD - 1))
                nc.vector.tensor_copy(decp[jt][:], ps[:])
            # z^T tiles + output matmul, one M-tile (=2 encoder rows) at a time
            for m in range(BT * U // 128):
                zt = [ztp.tile([128, 128], F32R, tag=f"zt{jt}", name=f"zt{jt}") for jt in range(4)]
                for jt in range(4):
                    for half in range(2):
                        bt = 2 * m + half
                        b = bt // TC
                        nc.scalar.activation(
                            zt[jt][:, half * 64:(half + 1) * 64],
                            decp[jt][:, b * 64:(b + 1) * 64],
                            AF.Tanh, bias=encp[jt][:, bt:bt + 1])
                for ob in range(4):
                    obs = slice(ob * 512, (ob + 1) * 512)
                    ps = pj.tile([128, 512], F32, tag="pj", name="pj")
                    for jt in range(4):
                        _mm_r(nc, ps[:], lhsT=zt[jt][:], rhs=wout[jt][:, obs],
                              start=(jt == 0), stop=(jt == 3))
                    o_sb = osbp.tile([128, 512], F32, tag="osb", name="osb")
                    nc.vector.tensor_tensor(o_sb[:], in0=ps[:], in1=bout_sb[:, obs],
                                            op=OP.add)
                    nc.sync.dma_start(out[128 * m:128 * (m + 1), obs], o_sb[:])

    nc.compile()
    return nc


def _prep_inputs(hs_pad, ys_in_pad, embed, W_ih0, W_hh0, b_ih0, b_hh0,
                 W_ih1, W_hh1, b_ih1, b_hh1, W_enc, b_enc, W_dec, W_out, b_out):
    f = np.float32
    tr = lambda a: np.ascontiguousarray(np.asarray(a).T, dtype=f)
    common = {
        "ys_idx": np.ascontiguousarray(np.asarray(ys_in_pad).T.reshape(-1),
                                       dtype=np.int32),
        "embed": np.ascontiguousarray(embed, dtype=f),
        "wih0T": tr(W_ih0), "whh0T": tr(W_hh0),
        "wih1T": tr(W_ih1), "whh1T": tr(W_hh1),
        "bih0": np.asarray(b_ih0, f), "bhh0": np.asarray(b_hh0, f),
        "bih1": np.asarray(b_ih1, f), "bhh1": np.asarray(b_hh1, f),
        "wencT": tr(W_enc), "wdecT": tr(W_dec), "woutT": tr(W_out),
        "benc": np.asarray(b_enc, f),
        "bout_bc": np.ascontiguousarray(
            np.broadcast_to(np.asarray(b_out, f)[None, :], (128, O))),
        "ones_d": np.ones(256, f),
        "ident_f": np.eye(128, dtype=f),
        "ident_b": np.eye(128).astype(np.dtype("bfloat16") if hasattr(np, "bfloat16")
                                      else __import__("ml_dtypes").bfloat16),
    }
    hs_np = np.asarray(hs_pad, f)
    in_maps = []
    for c in range(NCORES):
        m = dict(common)
        m["hs"] = np.ascontiguousarray(
            hs_np[:, c * TC:(c + 1) * TC, :].reshape(BT, E))
        in_maps.append(m)
    return in_maps


def _get_runner():
    """Build (once) a reusable jitted SPMD callable.

    Weights are replicated across the 8 cores (in_specs=P()); only hs and the
    output are sharded over the leading axis. This avoids the 8x concat +
    retrace of run_bass_kernel_spmd on every call.
    """
    if "runner" in _CACHE:
        return _CACHE["runner"]
    import jax
    from jax.sharding import Mesh, PartitionSpec as P
    from jax.experimental.shard_map import shard_map
    from concourse import bass2jax
    import concourse.mybir as mybir_

    nc = _CACHE.get("nc")
    if nc is None:
        nc = _CACHE["nc"] = build_nc()
    bass2jax.install_neuronx_cc_hook()

    pname = nc.partition_id_tensor.name if nc.partition_id_tensor else None
    in_names, out_names, out_avals = [], [], []
    for alloc in nc.m.functions[0].allocations:
        if not isinstance(alloc, mybir_.MemoryLocationSet):
            continue
        name = alloc.memorylocations[0].name
        if alloc.kind == "ExternalInput":
            if name != pname:
                in_names.append(name)
        elif alloc.kind == "ExternalOutput":
            out_names.append(name)
            shape = tuple(alloc.tensor_shape)
            out_avals.append(jax.core.ShapedArray(shape, mybir_.dt.np(alloc.dtype)))
    n_params = len(in_names)
    all_names = in_names + out_names
    if pname is not None:
        all_names = all_names + [pname]

    def _body(*args):
        operands = list(args)
        if pname is not None:
            operands.append(bass2jax.partition_id_tensor())
        outs = bass2jax._bass_exec_p.bind(
            *operands,
            out_avals=tuple(out_avals),
            in_names=tuple(all_names),
            out_names=tuple(out_names),
            lowering_input_output_aliases=(),
            sim_require_finite=True,
            sim_require_nnan=True,
            nc=nc,
        )
        return tuple(outs)

    devices = jax.devices()[:NCORES]
    mesh = Mesh(np.asarray(devices), ("core",))
    in_specs = tuple(P("core") if n == "hs" else P() for n in in_names)
    in_specs = in_specs + (P("core"),) * len(out_names)
    out_specs = (P("core"),) * len(out_names)
    fn = jax.jit(shard_map(_body, mesh=mesh, in_specs=in_specs,
                           out_specs=out_specs, check_rep=False))

    def _chain(n):
        def body_n(*args):
            ins, outbuf = args[:n_params], args[n_params]
            for _ in range(n):
                (outbuf,) = _body(*ins, outbuf)
            return (outbuf,)
        return jax.jit(shard_map(body_n, mesh=mesh, in_specs=in_specs,
                                 out_specs=out_specs, check_rep=False))

    runner = (fn, in_names, out_names, out_avals, mesh, _chain)
    _CACHE["runner"] = runner
    return runner


def _device_args(in_maps):
    """Assemble the jit arguments (host-side) for the runner."""
    fn, in_names, out_names, out_avals, mesh, _chain = _get_runner()
    args = []
    for n in in_names:
        if n == "hs":
            args.append(np.concatenate([m["hs"] for m in in_maps], axis=0))
        else:
            args.append(in_maps[0][n])
    for av in out_avals:
        args.append(np.zeros((NCORES * av.shape[0],) + av.shape[1:], av.dtype))
    return args


def kernel(**inputs) -> np.ndarray:
    fn, in_names, out_names, out_avals, mesh, _chain = _get_runner()
    in_maps = _prep_inputs(**inputs)
    args = _device_args(in_maps)
    outs = fn(*args)
    out = np.asarray(outs[0])  # (8*4096, 2048)
    return out.reshape(NCORES, B, TC, U, O).transpose(1, 0, 2, 3, 4).reshape(B, T, U, O)


if __name__ == "__main__":
    import time
    t0 = time.time()
    nc = build_nc(steps=int(sys.argv[1]) if len(sys.argv) > 1 else U)
    print(f"built ok in {time.time()-t0:.1f}s", flush=True)



# revision 2
# speedup vs baseline: 3.9140x; 3.9140x over previous
"""RNN-T decoder on 8 Trainium2 cores — flipped (gate-major) LSTM recurrence.

v2 strategy vs baseline:
  - The LSTM recurrence computes gates^T = W_hh @ h^T with W_hh STATIONARY
    (lhsT) and the tiny h streamed (N=4), instead of streaming all of W_hh
    as rhs every step. With colw=32 the weight loads are col-tiled across
    4 PE column groups (tile_position), quadrupling effective LDWEIGHTS
    bandwidth; everything stays fp32r.
  - Gate-major layout makes ALL elementwise work [128, 32]-shaped (full
    partition utilization) and eliminates the per-step h transposes.
  - Joint network unchanged: sharded over T (16 columns per core).

kernel(**inputs) takes full unsharded inputs, returns (B, T, U, ODIM) f32.
"""
import sys
import numpy as np

sys.path.insert(0, "/opt/trn_rl_repo")

import concourse.bass as bass
import concourse.bacc as bacc
import concourse.mybir as mybir
import concourse.tile as tile
from contextlib import ExitStack

F32 = mybir.dt.float32
F32R = mybir.dt.float32r
BF16 = mybir.dt.bfloat16
FP8 = mybir.dt.float8e4
I32 = mybir.dt.int32
AF = mybir.ActivationFunctionType
OP = mybir.AluOpType

B, T, U = 4, 128, 64
NCORES = 8
TC = T // NCORES
E, D, G = 512, 1024, 4096
J, O = 512, 2048
UB = U * B
BT = B * TC
KD = D // 128
TYPE_ORDER = [2, 0, 1, 3]  # g, i, f, o (PyTorch gate order i,f,g,o on the G axis)

_CACHE = {}


def _mm_r(nc, out, lhsT, rhs, **kw):
    nc.tensor.matmul(out, lhsT=lhsT.bitcast(F32R), rhs=rhs.bitcast(F32R), **kw)


def _emit_xproj(nc, pools, rhs_of_ec, w_of_ec, nchunks, bih, bhh, gx_all,
                accum, with_bias, steps):
    """gx_all[:, 128u + 4m + b] (+)= (W @ x^T) + bias, for 32 m-blocks.

    rhs_of_ec: ec -> [128, 256] rhs tile (x^T chunk, K on partitions,
               token cols 4u+b)
    w_of_ec:   ec -> [128, G] tile of W.T rows for that chunk (lhsT source)
    """
    pbig, bpool = pools["pbig"], pools["bias"]
    ones_r = pools["ones"]
    gx_v = gx_all[:].rearrange("p (u m b) -> p m u b", u=steps, m=32, b=4)
    for gm in range(32):
        if with_bias and gm % 4 == 0:
            cb = slice((gm // 4) * 512, (gm // 4) * 512 + 512)
            ba = bpool.tile([1, 512], F32R, tag="ba", name="ba")
            bb = bpool.tile([1, 512], F32, tag="bb", name="bb")
            nc.sync.dma_start(ba[:1, :], bih[None, cb].bitcast(F32R))
            nc.sync.dma_start(bb[:1, :], bhh[None, cb])
            nc.vector.tensor_tensor(ba[:1, :], in0=ba[:1, :],
                                    in1=bb[:1, :], op=OP.add)
        ps = pbig.tile([128, 256], F32, tag="pbig", name="pbig")
        for ec in range(nchunks):
            _mm_r(nc, ps[:],
                  lhsT=w_of_ec(ec)[:, 128 * gm: 128 * (gm + 1)],
                  rhs=rhs_of_ec(ec),
                  start=(ec == 0), stop=(ec == nchunks - 1 and not with_bias))
        if with_bias:
            gs = slice(128 * (gm % 4), 128 * (gm % 4) + 128)
            _mm_r(nc, ps[:], lhsT=ba[:1, gs], rhs=ones_r[:1, :256],
                  start=False, stop=True)
        out_ap = gx_v[:, gm]
        in_ap = ps[:].rearrange("p (u b) -> p u b", u=steps, b=4)
        if accum:
            nc.vector.tensor_tensor(out_ap, in0=out_ap, in1=in_ap, op=OP.add)
        else:
            nc.vector.tensor_copy(out_ap, in_ap)


def _emit_lstm_layer_flip(nc, pools, whh, gx_all, hist, c_sb, steps,
                          colw, ncg, wdt):
    """One LSTM layer, gate-major. whh: 8 SBUF [128, G] tiles (W_hh.T chunks).

    hist: KD tiles [128, 4*(U+1)] — col block u+1 = h[u]^T chunk (block 0
    zeros); written here, consumed by the next layer's x-proj / joint.

    PSUM: i/f/g gates and the o gate live in separate banks per column
    group, so the g/i/f tail reads never touch a bank the PE is still
    writing (PE-write + DVE-read of one bank is a fatal HW collision).
    """
    pgates = pools["pgates"]
    gpre, gact = pools["gpre"], pools["gact"]
    tc_t, fc, ig = pools["tc"], pools["fc"], pools["ig"]
    h_sb = pools["h_sb"]
    h_rhs = pools["h_rhs"]  # dtype-matched rhs view source (may be h_sb)
    njs = 1 if colw == 128 else 4
    for u in range(steps):
        if colw == 128:
            pifg = [pgates.tile([128, 96], F32, tag="pifg", name="pifg")]
            po = [pgates.tile([128, 32], F32, tag="po", name="po")]
        else:
            pifg = [pgates.tile([128, 96], F32, tag=f"pifg{j}", name=f"pifg{j}")
                    for j in range(4)]
            po = [pgates.tile([128, 32], F32, tag=f"po{j}", name=f"po{j}")
                  for j in range(4)]

        def emit_type(t):
            for M in range(8 * t, 8 * t + 8):
                dst, c0 = (po, 4 * (M - 24)) if t == 3 else (pifg, 4 * M)
                if colw == 128:
                    for k in range(KD):
                        rhs = h_rhs[:, 4 * k:4 * k + 4]
                        if wdt is F32R:
                            rhs = rhs.bitcast(F32R)
                        nc.tensor.matmul(
                            dst[0][:, c0:c0 + 4],
                            lhsT=whh[k][:, 128 * M:128 * (M + 1)],
                            rhs=rhs,
                            start=(k == 0), stop=(k == KD - 1))
                else:
                    for k in range(KD):
                        for j in range(4):
                            _mm_r(nc, dst[j][32 * j:32 * j + 32, c0:c0 + 4],
                                  lhsT=whh[k][:, 128 * M + 32 * j:
                                              128 * M + 32 * j + 32],
                                  rhs=h_rhs[:, 4 * k:4 * k + 4],
                                  tile_position=(0, 32 * j),
                                  start=(k == 0), stop=(k == KD - 1))

        def tail_type(t):
            ts = slice(32 * t, 32 * t + 32)
            src, c0 = (po, 0) if t == 3 else (pifg, 32 * t)
            for j in range(njs):
                js = slice(0, 128) if njs == 1 else slice(32 * j, 32 * j + 32)
                nc.vector.tensor_tensor(
                    gpre[js, ts], in0=src[j if njs > 1 else 0][js, c0:c0 + 32],
                    in1=gx_all[js, 128 * u + 32 * t:128 * u + 32 * t + 32],
                    op=OP.add)
            nc.scalar.activation(gact[:, ts], gpre[:, ts],
                                 AF.Tanh if t == 2 else AF.Sigmoid)

        for t in (2, 0, 1):  # g, i, f
            emit_type(t)
        for t in (2, 0, 1):
            tail_type(t)
        # c chain overlaps o's matmuls
        nc.vector.tensor_tensor(fc[:], in0=gact[:, 32:64], in1=c_sb[:],
                                op=OP.mult)
        nc.vector.tensor_tensor(ig[:], in0=gact[:, 0:32],
                                in1=gact[:, 64:96], op=OP.mult)
        nc.vector.tensor_tensor(c_sb[:], in0=fc[:], in1=ig[:], op=OP.add)
        nc.scalar.activation(tc_t[:], c_sb[:], AF.Tanh)
        emit_type(3)
        tail_type(3)
        nc.vector.tensor_tensor(h_sb[:], in0=gact[:, 96:128], in1=tc_t[:],
                                op=OP.mult)
        if h_rhs is not h_sb:
            nc.vector.tensor_copy(h_rhs[:], h_sb[:])
        for k in range(KD):
            nc.vector.tensor_copy(
                hist[k][:, 4 * (u + 1):4 * (u + 1) + 4],
                h_sb[:, 4 * k:4 * k + 4])


def build_nc(steps=U, layers=2, joint=True, colw=128, ncg=4, wdtype="bf16",
             pg_bufs=3):
    wdt = {"f32r": F32R, "bf16": BF16, "fp8": FP8}[wdtype]
    nc = bacc.Bacc("TRN2", target_bir_lowering=False, debug=False)

    hs = nc.dram_tensor("hs", [BT, E], F32, kind="ExternalInput")
    ys_idx = nc.dram_tensor("ys_idx", [UB], I32, kind="ExternalInput")
    embed = nc.dram_tensor("embed", [O, E], F32, kind="ExternalInput")
    wih0T = nc.dram_tensor("wih0T", [E, G], F32R, kind="ExternalInput")
    whh0T = nc.dram_tensor("whh0T", [D, G], wdt, kind="ExternalInput")
    wih1T = nc.dram_tensor("wih1T", [D, G], F32R, kind="ExternalInput")
    whh1T = nc.dram_tensor("whh1T", [D, G], wdt, kind="ExternalInput")
    bih0 = nc.dram_tensor("bih0", [G], F32, kind="ExternalInput")
    bhh0 = nc.dram_tensor("bhh0", [G], F32, kind="ExternalInput")
    bih1 = nc.dram_tensor("bih1", [G], F32, kind="ExternalInput")
    bhh1 = nc.dram_tensor("bhh1", [G], F32, kind="ExternalInput")
    wencT = nc.dram_tensor("wencT", [E, J], F32R, kind="ExternalInput")
    wdecT = nc.dram_tensor("wdecT", [D, J], F32R, kind="ExternalInput")
    woutT = nc.dram_tensor("woutT", [J, O], F32R, kind="ExternalInput")
    benc = nc.dram_tensor("benc", [J], F32R, kind="ExternalInput")
    bout_bc = nc.dram_tensor("bout_bc", [128, O], F32, kind="ExternalInput")
    ones_d = nc.dram_tensor("ones_d", [256], F32R, kind="ExternalInput")
    ident_f = nc.dram_tensor("ident_f", [128, 128], F32, kind="ExternalInput")
    out = nc.dram_tensor("out", [BT * U, O], F32, kind="ExternalOutput")

    with tile.TileContext(nc) as tc, ExitStack() as es:
        cpool = es.enter_context(tc.tile_pool(name="const", bufs=1))
        ppool = es.enter_context(tc.tile_pool(name="persist", bufs=1))

        ident = cpool.tile([128, 128], F32, tag="ident", name="ident")
        nc.sync.dma_start(ident[:], ident_f[:])
        ones_r = cpool.tile([1, 256], F32R, tag="ones", name="ones")
        nc.sync.dma_start(ones_r[:1, :], ones_d[None, :])

        gx_all = ppool.tile([128, 128 * U], F32, tag="gx", name="gx")
        hT0 = [ppool.tile([128, 4 * (U + 1)], F32R, tag=f"hT0_{k}",
                          name=f"hT0_{k}") for k in range(KD)]
        hT1 = [ppool.tile([128, 4 * (U + 1)], F32R, tag=f"hT1_{k}",
                          name=f"hT1_{k}") for k in range(KD)]
        c_sb = ppool.tile([128, 32], F32, tag="c", name="c")
        h_sb = ppool.tile([128, 32], F32, tag="h", name="h")
        gpre = ppool.tile([128, 128], F32, tag="gpre", name="gpre")
        gact = ppool.tile([128, 128], F32, tag="gact", name="gact")
        tc_t = ppool.tile([128, 32], F32, tag="tc", name="tc")
        fc = ppool.tile([128, 32], F32, tag="fcs", name="fcs")
        ig = ppool.tile([128, 32], F32, tag="igs", name="igs")
        if wdt is F32R:
            h_rhs = h_sb
        else:
            h_rhs = ppool.tile([128, 32], wdt, tag="hr", name="hr")
        for k in range(KD):
            nc.gpsimd.memset(hT0[k][:, :4].bitcast(F32), 0.0)
            nc.gpsimd.memset(hT1[k][:, :4].bitcast(F32), 0.0)
        nc.gpsimd.memset(c_sb[:], 0.0)
        nc.gpsimd.memset(h_sb[:], 0.0)
        if h_rhs is not h_sb:
            nc.gpsimd.memset(h_rhs[:].bitcast(F32) if wdt is F32R else h_rhs[:],
                             0.0)

        pools = {"ones": ones_r, "gpre": gpre, "gact": gact, "tc": tc_t,
                 "fc": fc, "ig": ig, "h_sb": h_sb, "h_rhs": h_rhs}

        wenc = [ppool.tile([128, J], F32R, tag=f"wenc{ec}", name=f"wenc{ec}")
                for ec in range(4)]
        benc_sb = ppool.tile([1, J], F32R, tag="benc", name="benc")
        hs_sb = ppool.tile([BT, E], F32, tag="hs_sb", name="hs_sb")
        hsT = [ppool.tile([128, BT], F32R, tag=f"hsT{ec}", name=f"hsT{ec}")
               for ec in range(4)]
        encp = [ppool.tile([128, BT], F32, tag=f"encp{jt}", name=f"encp{jt}")
                for jt in range(4)]

        # ---- Phase 1+2: embedding gather -> eys^T, layer-0 x-projection ----
        with tc.tile_pool(name="ph2", bufs=1) as p2, \
             tc.tile_pool(name="bias2", bufs=2) as bpool2, \
             tc.tile_pool(name="pbig", bufs=4, space="PSUM") as pbig, \
             tc.tile_pool(name="pT", bufs=2, space="PSUM") as pT:
            pools["pbig"] = pbig
            pools["bias"] = bpool2
            eysT = [p2.tile([128, 256], F32R, tag=f"eysT{ec}", name=f"eysT{ec}")
                    for ec in range(4)]
            idxs = []
            for t in range(2):
                idx = p2.tile([128, 1], I32, tag=f"idx{t}", name=f"idx{t}")
                nc.sync.dma_start(idx[:, :1], ys_idx[128 * t:128 * (t + 1), None])
                idxs.append(idx)
            wih0 = [p2.tile([128, G], F32R, tag=f"wih0_{ec}", name=f"wih0_{ec}")
                    for ec in range(4)]
            for ec in range(4):
                nc.sync.dma_start(wih0[ec][:], wih0T[128 * ec:128 * (ec + 1), :])
            for t in range(2):
                idx = idxs[t]
                ey = p2.tile([128, E], F32, tag=f"ey{t}", name=f"ey{t}")
                nc.gpsimd.indirect_dma_start(
                    out=ey[:], out_offset=None, in_=embed[:],
                    in_offset=bass.IndirectOffsetOnAxis(ap=idx[:, :1], axis=0))
                for ec in range(4):
                    tp = pT.tile([128, 128], F32, tag="pT", name="pT")
                    nc.tensor.transpose(tp[:], in_=ey[:, 128 * ec:128 * (ec + 1)],
                                        identity=ident[:])
                    nc.vector.tensor_copy(eysT[ec][:, 128 * t:128 * (t + 1)],
                                          tp[:])
            _emit_xproj(nc, pools, lambda ec: eysT[ec][:], lambda ec: wih0[ec],
                        4, bih0, bhh0, gx_all, accum=False, with_bias=True,
                        steps=U)

            for ec in range(4):
                nc.sync.dma_start(wenc[ec][:], wencT[128 * ec:128 * (ec + 1), :])
            nc.sync.dma_start(hs_sb[:], hs[:])
            nc.sync.dma_start(benc_sb[:1, :], benc[None, :])
            for ec in range(4):
                tp = pT.tile([128, 128], F32, tag="pT", name="pT")
                nc.tensor.transpose(tp[:, :BT],
                                    in_=hs_sb[:, 128 * ec:128 * (ec + 1)],
                                    identity=ident[:BT, :BT])
                nc.vector.tensor_copy(hsT[ec][:], tp[:, :BT])
            for jt in range(4):
                tpp = pT.tile([128, 128], F32, tag="pT", name="pT")
                pse = tpp[:, :BT]
                for ec in range(4):
                    _mm_r(nc, pse, lhsT=wenc[ec][:, 128 * jt:128 * (jt + 1)],
                          rhs=hsT[ec][:], start=(ec == 0), stop=False)
                _mm_r(nc, pse, lhsT=benc_sb[:1, 128 * jt:128 * (jt + 1)],
                      rhs=ones_r[:1, :BT], start=False, stop=True)
                nc.vector.tensor_copy(encp[jt][:], pse)

        # ---- Phase 3: layer-0 recurrence ----
        with tc.tile_pool(name="whhA", bufs=1) as whhp, \
             tc.tile_pool(name="pgatesA", bufs=pg_bufs, space="PSUM") as pgates:
            pools["pgates"] = pgates
            whh = [whhp.tile([128, G], wdt, tag=f"whh{k}", name=f"whh{k}")
                   for k in range(KD)]
            for k in range(KD):
                nc.sync.dma_start(whh[k][:], whh0T[128 * k:128 * (k + 1), :])
            _emit_lstm_layer_flip(nc, pools, whh, gx_all, hT0, c_sb, steps,
                                  colw, ncg, wdt)

        # ---- Phase 4: layer-1 x-projection (streams W_ih1, accumulates) ----
        with tc.tile_pool(name="ph4", bufs=1) as p4, \
             tc.tile_pool(name="bias4", bufs=2) as bpool4, \
             tc.tile_pool(name="pbig2", bufs=4, space="PSUM") as pbig2:
            pools["pbig"] = pbig2
            pools["bias"] = bpool4
            wbuf = [p4.tile([128, G], F32R, tag=f"wih1_{i}", name=f"wih1_{i}")
                    for i in range(4)]
            for p in range(4):
                for i in range(2):
                    k = 2 * p + i
                    nc.sync.dma_start(wbuf[(2 * p + i) % 4][:],
                                      wih1T[128 * k:128 * (k + 1), :])
                _emit_xproj(nc, pools,
                            lambda i, p=p: hT0[2 * p + i][:, 4:4 + 4 * U],
                            lambda i, p=p: wbuf[(2 * p + i) % 4], 2, bih1, bhh1,
                            gx_all, accum=(p > 0), with_bias=(p == 3), steps=U)

        # ---- Phase 5: layer-1 recurrence ----
        nc.gpsimd.memset(c_sb[:], 0.0)
        nc.gpsimd.memset(h_sb[:], 0.0)
        if h_rhs is not h_sb:
            nc.gpsimd.memset(h_rhs[:], 0.0)
        if layers < 2:
            for k in range(KD):
                nc.vector.tensor_copy(hT1[k][:, 4:4 + 4 * steps],
                                      hT0[k][:, 4:4 + 4 * steps])
        with tc.tile_pool(name="whhB", bufs=1) as whhp2, \
             tc.tile_pool(name="pgatesB", bufs=pg_bufs, space="PSUM") as pgates2:
            pools["pgates"] = pgates2
            if layers >= 2:
                whh2 = [whhp2.tile([128, G], wdt, tag=f"whh2_{k}",
                                   name=f"whh2_{k}") for k in range(KD)]
                for k in range(KD):
                    nc.sync.dma_start(whh2[k][:], whh1T[128 * k:128 * (k + 1), :])
                _emit_lstm_layer_flip(nc, pools, whh2, gx_all, hT1, c_sb,
                                      steps, colw, ncg, wdt)

        # ---- Phase 6: joint network on this core's T-slice ----
        if joint:
            with tc.tile_pool(name="joint", bufs=1) as jp, \
                 tc.tile_pool(name="zt", bufs=4) as ztp, \
                 tc.tile_pool(name="osb", bufs=4) as osbp, \
                 tc.tile_pool(name="pj", bufs=4, space="PSUM") as pj:
                wdec = [jp.tile([128, J], F32R, tag=f"wdec{k}", name=f"wdec{k}")
                        for k in range(KD)]
                wout = [jp.tile([128, O], F32R, tag=f"wout{jt}", name=f"wout{jt}")
                        for jt in range(4)]
                bout_sb = jp.tile([128, O], F32, tag="bout", name="bout")
                decp = [jp.tile([128, 256], F32, tag=f"decp{jt}", name=f"decp{jt}")
                        for jt in range(4)]
                for k in range(KD):
                    nc.sync.dma_start(wdec[k][:], wdecT[128 * k:128 * (k + 1), :])
                nc.sync.dma_start(bout_sb[:], bout_bc[:])
                for jt in range(4):
                    nc.sync.dma_start(wout[jt][:], woutT[128 * jt:128 * (jt + 1), :])

                for jt in range(4):
                    ps = pj.tile([128, 256], F32, tag="pj", name="pj")
                    for k in range(KD):
                        rhs = hT1[k][:, 4:4 + 4 * U].rearrange(
                            "p (u b) -> p b u", u=U, b=B)
                        _mm_r(nc, ps[:], lhsT=wdec[k][:, 128 * jt:128 * (jt + 1)],
                              rhs=rhs, start=(k == 0), stop=(k == KD - 1))
                    nc.vector.tensor_copy(decp[jt][:], ps[:])
                for m in range(BT * U // 128):
                    zt = [ztp.tile([128, 128], F32R, tag=f"zt{jt}", name=f"zt{jt}")
                          for jt in range(4)]
                    for jt in range(4):
                        for half in range(2):
                            bt = 2 * m + half
                            b = bt // TC
                            nc.scalar.activation(
                                zt[jt][:, half * 64:(half + 1) * 64],
                                decp[jt][:, b * 64:(b + 1) * 64],
                                AF.Tanh, bias=encp[jt][:, bt:bt + 1])
                    for ob in range(4):
                        obs = slice(ob * 512, (ob + 1) * 512)
                        ps = pj.tile([128, 512], F32, tag="pj", name="pj")
                        for jt in range(4):
                            _mm_r(nc, ps[:], lhsT=zt[jt][:], rhs=wout[jt][:, obs],
                                  start=(jt == 0), stop=(jt == 3))
                        o_sb = osbp.tile([128, 512], F32, tag="osb", name="osb")
                        nc.vector.tensor_tensor(o_sb[:], in0=ps[:],
                                                in1=bout_sb[:, obs], op=OP.add)
                        nc.sync.dma_start(out[128 * m:128 * (m + 1), obs], o_sb[:])
        else:
            zsrc = ppool.tile([128, 512], F32, tag="zsrc", name="zsrc")
            nc.gpsimd.memset(zsrc[:], 0.0)
            for m0 in range(BT * U // 128):
                for ob in range(4):
                    nc.sync.dma_start(out[128 * m0:128 * (m0 + 1),
                                          ob * 512:(ob + 1) * 512], zsrc[:])

    nc.compile()
    return nc


def _prep_inputs(hs_pad, ys_in_pad, embed, W_ih0, W_hh0, b_ih0, b_hh0,
                 W_ih1, W_hh1, b_ih1, b_hh1, W_enc, b_enc, W_dec, W_out, b_out,
                 wdtype="bf16"):
    f = np.float32
    tr = lambda a: np.ascontiguousarray(np.asarray(a).T, dtype=f)
    if wdtype == "f32r":
        wcast = lambda a: a
    else:
        import ml_dtypes
        wdt_np = (ml_dtypes.bfloat16 if wdtype == "bf16"
                  else ml_dtypes.float8_e4m3fn)
        wcast = lambda a: np.ascontiguousarray(a.astype(wdt_np))
    common = {
        "ys_idx": np.ascontiguousarray(np.asarray(ys_in_pad).T.reshape(-1),
                                       dtype=np.int32),
        "embed": np.ascontiguousarray(embed, dtype=f),
        "wih0T": tr(W_ih0), "whh0T": wcast(tr(W_hh0)),
        "wih1T": tr(W_ih1), "whh1T": wcast(tr(W_hh1)),
        "bih0": np.asarray(b_ih0, f), "bhh0": np.asarray(b_hh0, f),
        "bih1": np.asarray(b_ih1, f), "bhh1": np.asarray(b_hh1, f),
        "wencT": tr(W_enc), "wdecT": tr(W_dec), "woutT": tr(W_out),
        "benc": np.asarray(b_enc, f),
        "bout_bc": np.ascontiguousarray(
            np.broadcast_to(np.asarray(b_out, f)[None, :], (128, O))),
        "ones_d": np.ones(256, f),
        "ident_f": np.eye(128, dtype=f),
    }
    hs_np = np.asarray(hs_pad, f)
    in_maps = []
    for c in range(NCORES):
        m = dict(common)
        m["hs"] = np.ascontiguousarray(
            hs_np[:, c * TC:(c + 1) * TC, :].reshape(BT, E))
        in_maps.append(m)
    return in_maps


def _get_runner():
    if "runner" in _CACHE:
        return _CACHE["runner"]
    import jax
    from jax.sharding import Mesh, PartitionSpec as P
    from jax.experimental.shard_map import shard_map
    from concourse import bass2jax
    import concourse.mybir as mybir_

    nc = _CACHE.get("nc")
    if nc is None:
        nc = _CACHE["nc"] = build_nc(**_CACHE.get("build_kw", {}))
    bass2jax.install_neuronx_cc_hook()

    pname = nc.partition_id_tensor.name if nc.partition_id_tensor else None
    in_names, out_names, out_avals = [], [], []
    for alloc in nc.m.functions[0].allocations:
        if not isinstance(alloc, mybir_.MemoryLocationSet):
            continue
        name = alloc.memorylocations[0].name
        if alloc.kind == "ExternalInput":
            if name != pname:
                in_names.append(name)
        elif alloc.kind == "ExternalOutput":
            out_names.append(name)
            shape = tuple(alloc.tensor_shape)
            out_avals.append(jax.core.ShapedArray(shape, mybir_.dt.np(alloc.dtype)))
    n_params = len(in_names)
    all_names = in_names + out_names
    if pname is not None:
        all_names = all_names + [pname]

    def _body(*args):
        operands = list(args)
        if pname is not None:
            operands.append(bass2jax.partition_id_tensor())
        outs = bass2jax._bass_exec_p.bind(
            *operands,
            out_avals=tuple(out_avals),
            in_names=tuple(all_names),
            out_names=tuple(out_names),
            lowering_input_output_aliases=(),
            sim_require_finite=True,
            sim_require_nnan=True,
            nc=nc,
        )
        return tuple(outs)

    devices = jax.devices()[:NCORES]
    mesh = Mesh(np.asarray(devices), ("core",))
    in_specs = tuple(P("core") if n == "hs" else P() for n in in_names)
    in_specs = in_specs + (P("core"),) * len(out_names)
    out_specs = (P("core"),) * len(out_names)
    fn = jax.jit(shard_map(_body, mesh=mesh, in_specs=in_specs,
                           out_specs=out_specs, check_rep=False))

    def _chain(n):
        def body_n(*args):
            ins, outbuf = args[:n_params], args[n_params]
            for _ in range(n):
                (outbuf,) = _body(*ins, outbuf)
            return (outbuf,)
        return jax.jit(shard_map(body_n, mesh=mesh, in_specs=in_specs,
                                 out_specs=out_specs, check_rep=False))

    runner = (fn, in_names, out_names, out_avals, mesh, _chain)
    _CACHE["runner"] = runner
    return runner


def _device_args(in_maps):
    fn, in_names, out_names, out_avals, mesh, _chain = _get_runner()
    args = []
    for n in in_names:
        if n == "hs":
            args.append(np.concatenate([m["hs"] for m in in_maps], axis=0))
        else:
            args.append(in_maps[0][n])
    for av in out_avals:
        args.append(np.zeros((NCORES * av.shape[0],) + av.shape[1:], av.dtype))
    return args


def kernel(**inputs) -> np.ndarray:
    fn, in_names, out_names, out_avals, mesh, _chain = _get_runner()
    in_maps = _prep_inputs(**inputs,
                           wdtype=_CACHE.get("build_kw", {}).get("wdtype", "bf16"))
    args = _device_args(in_maps)
    outs = fn(*args)
    out = np.asarray(outs[0])
    return out.reshape(NCORES, B, TC, U, O).transpose(1, 0, 2, 3, 4).reshape(B, T, U, O)


if __name__ == "__main__":
    import time
    t0 = time.time()
    nc = build_nc(steps=int(sys.argv[1]) if len(sys.argv) > 1 else U)
    print(f"built ok in {time.time()-t0:.1f}s", flush=True)
